# revision 18
# baseline (speedup 1.0000x reference)
"""Trainium2 Bass kernel for nn_MissingValueImputer (GAT-based imputer).

Mathematical structure of the reference model: the feature projection
broadcasts the *same* projected sample vector to every one of the N graph
nodes, so the per-head node features h[b,h,n,:] do not depend on n. The
additive attention logits e[b,h,i,j] are therefore constant over (i,j),
and the adjacency-masked softmax collapses to attn = adj / k (each top-k
adjacency row has exactly k ones, and softmax of a constant masked row is
uniform). Message passing then yields h' = relu(h * sum_j attn[i,j]) =
relu(h) for every node, so the whole GAT

    imputed = mean_n( concat_heads(h') @ op_W + op_b )

reduces exactly (up to float reassociation) to a 2-layer MLP applied to
each sample vector:

    G       = x_flat @ (fp_W @ Wcat) + fp_b @ Wcat      # [BW, H*F]
    imputed = relu(G) @ op_W + op_b                      # [BW, N]
    output  = x * (1 - mask) + imputed * mask

with Wcat[f, h*F + o] = gat_W[h, o, f]. The adjacency matrix (second
return value) depends only on node_emb and is computed with the exact
reference jax ops on CPU. The MLP + masked mix runs on 8 NeuronCores,
data-parallel over the BW = 512 batch rows (64 rows per core), with all
on-device tensors kept transposed so the partition dimension is 128.

Device layout (per core, everything [128 partitions, cols]):
  inA  [128,133] = xT(64) | maskT(64) | cb(4) | opb(1)   - sync HWDGE ring
  opW  [128,512] = op_W K-chunks side by side             - sync HWDGE ring
  Mw   [128,512] = fp_W @ Wcat                            - scalar HWDGE ring
  GT_j = Mw[:,j*128:...].T @ xT          (PE, psum)
  rT_j = max(GT_j + cb[:,j], 0)          (DVE tensor_scalar, no ACT table)
  impT = sum_j opW[:,j*128:...].T @ rT_j (PE, psum accumulate)
  outT = xT + maskT * (impT + opb - xT)  (DVE x3)
"""

import numpy as np

B, W, N, F, H = 8, 64, 128, 128, 4
BW = B * W            # 512 batch rows
NCORES = 8
ROWS = BW // NCORES   # 64 rows per core
HID = H * F           # 512 hidden units

# packed input layout, all [128, cols]:
#   xT(64) | uT(64) | mT(64) | cb(4) | opb(1) | Mw(512) | opW(512)
# HW-measured: keeping the epilogue tensors (uT, mT) in the FIRST piece beats
# deferring them to the last piece (7.4 vs 11.1 us/iter).
C_XT = 0
C_UT = ROWS
C_MT = 2 * ROWS
C_CB = 3 * ROWS
C_OPB = C_CB + H
C_MW = C_OPB + 1                  # 197
C_OPW = C_MW + HID                # 709
C_TOT = C_OPW + HID               # 1221

# DMA split: list of (engine, col_start, col_end) pieces of the packed input.
# HW-measured best: small piece on sync, Mw on the scalar HWDGE ring, opW on
# sync (8.3us/iter vs 9.5 for two-piece and 10.4 for one-piece).
DEFAULT_SPLIT = (("sync", 0, C_MW), ("scalar", C_MW, C_OPW), ("sync", C_OPW, C_TOT))

_cache = {}


def _build_adjacency_host(node_emb, k):
    """Exact replica of the reference build_adjacency, on jax-CPU."""
    import jax
    import jax.numpy as jnp

    cpu = jax.devices("cpu")[0]
    with jax.default_device(cpu):
        ne = jnp.asarray(np.asarray(node_emb, np.float32))
        norm = ne / jnp.linalg.norm(ne, axis=1, keepdims=True)
        sim = norm @ norm.T
        _, idx = jax.lax.top_k(sim, k)
        n = sim.shape[0]
        rows = jnp.arange(n)[:, None]
        adj = jnp.zeros_like(sim).at[rows, idx].set(1.0)
    return np.asarray(adj)


def _build_bass(iters=1, loop_n=0, split=DEFAULT_SPLIT, out_engine="split",
                scratch=16384, relu_engine="vector"):
    import concourse.mybir as mybir
    from concourse import bacc
    from concourse.alu_op_type import AluOpType
    from concourse.tile import TileContext

    dt = mybir.dt.float32
    nc = bacc.Bacc(
        "TRN2",
        target_bir_lowering=False,
        debug=False,
        enable_asserts=True,
        num_devices=NCORES,
        dynamic_dma_scratch_size=scratch,
    )

    bigin = nc.dram_tensor("bigin", [N, C_TOT], dt, kind="ExternalInput").ap()
    outT = nc.dram_tensor("outT", [N, ROWS], dt, kind="ExternalOutput").ap()

    with TileContext(nc) as tc:
        with (
            tc.tile_pool(name="sb", bufs=1) as sb,
            tc.tile_pool(name="ps", bufs=1, space="PSUM") as ps,
        ):
            def body(_iv=None):
                big_s = sb.tile([N, C_TOT], dt, tag="big")
                for eng, c0, c1 in split:
                    dma = nc.sync if eng == "sync" else nc.scalar
                    dma.dma_start(out=big_s[:, c0:c1], in_=bigin[:, c0:c1])

                xT_ap = big_s[:, C_XT : C_XT + ROWS]
                uT_ap = big_s[:, C_UT : C_UT + ROWS]
                mT_ap = big_s[:, C_MT : C_MT + ROWS]
                opb_ap = big_s[:, C_OPB : C_OPB + 1]

                imp_ps = ps.tile([N, ROWS], mybir.dt.float32, tag="imp")
                for j in range(H):
                    # GT_j = Mw[:, j-chunk].T @ xT  -> [128 hidden, 64 rows]
                    gT_ps = ps.tile([F, ROWS], mybir.dt.float32, tag=f"g{j}")
                    nc.tensor.matmul(
                        gT_ps[:],
                        big_s[:, C_MW + j * F : C_MW + (j + 1) * F],
                        xT_ap,
                        start=True,
                        stop=True,
                    )
                    # rT_j = max(GT_j + cb_j, 0)
                    rT_s = sb.tile([F, ROWS], dt, tag=f"r{j}")
                    if relu_engine == "vector":
                        nc.vector.tensor_scalar(
                            rT_s[:],
                            gT_ps[:],
                            big_s[:, C_CB + j : C_CB + j + 1],
                            0.0,
                            AluOpType.add,
                            AluOpType.max,
                        )
                    else:
                        nc.scalar.activation(
                            rT_s[:],
                            gT_ps[:],
                            mybir.ActivationFunctionType.Relu,
                            bias=big_s[:, C_CB + j : C_CB + j + 1],
                            scale=1.0,
                        )
                    # impT += opW_j.T @ rT_j  -> [128 out, 64 rows]
                    nc.tensor.matmul(
                        imp_ps[:],
                        big_s[:, C_OPW + j * F : C_OPW + (j + 1) * F],
                        rT_s[:],
                        start=(j == 0),
                        stop=(j == H - 1),
                    )

                # outT = uT + maskT * (impT + opb),  uT = (x*(1-mask)).T
                p_s = sb.tile([N, ROWS], dt, tag="p")
                o_s = sb.tile([N, ROWS], dt, tag="o")
                if out_engine == "split4":
                    q = ROWS // 4
                    for i, odma in enumerate((nc.sync, nc.scalar, nc.sync, nc.scalar)):
                        hs = slice(i * q, (i + 1) * q)
                        nc.vector.scalar_tensor_tensor(
                            p_s[:, hs], imp_ps[:, hs], opb_ap, mT_ap[:, hs],
                            AluOpType.add, AluOpType.mult,
                        )
                        nc.vector.tensor_add(o_s[:, hs], p_s[:, hs], uT_ap[:, hs])
                        odma.dma_start(out=outT[:, hs], in_=o_s[:, hs])
                elif out_engine.startswith("split"):
                    eng = {"split": (nc.sync, nc.scalar),
                           "split-ss": (nc.sync, nc.sync),
                           "split-aa": (nc.scalar, nc.scalar)}[out_engine]
                    halves = ((slice(0, ROWS // 2), eng[0]),
                              (slice(ROWS // 2, ROWS), eng[1]))
                    for hs, odma in halves:
                        nc.vector.scalar_tensor_tensor(
                            p_s[:, hs], imp_ps[:, hs], opb_ap, mT_ap[:, hs],
                            AluOpType.add, AluOpType.mult,
                        )
                        nc.vector.tensor_add(o_s[:, hs], p_s[:, hs], uT_ap[:, hs])
                        odma.dma_start(out=outT[:, hs], in_=o_s[:, hs])
                else:
                    nc.vector.scalar_tensor_tensor(
                        p_s[:], imp_ps[:], opb_ap, mT_ap,
                        AluOpType.add, AluOpType.mult,
                    )
                    nc.vector.tensor_add(o_s[:], p_s[:], uT_ap)
                    odma = nc.sync if out_engine == "sync" else nc.scalar
                    odma.dma_start(out=outT[:], in_=o_s[:])

            if loop_n:
                tc.For_i_unrolled(0, loop_n, 1, body, max_unroll=8)
            else:
                for _ in range(iters):
                    body()

    nc.compile()
    return nc


def _pack_weights(gat_W, fp_W, fp_b, op_W, op_b):
    # Wcat[f, h*F+o] = gat_W[h, o, f]; fold the (linear) feature projection in.
    Wcat = np.ascontiguousarray(np.transpose(gat_W, (2, 0, 1)).reshape(F, HID))
    Mw = np.ascontiguousarray(fp_W @ Wcat)                 # [128, 512]
    c = fp_b @ Wcat                                        # [512]
    cb = np.ascontiguousarray(c.reshape(H, F).T)           # [128, 4]
    # opW packed so that col block j holds op_W rows j*128:(j+1)*128
    opWp = np.ascontiguousarray(
        op_W.reshape(H, F, N).transpose(1, 0, 2).reshape(F, HID)
    )
    return Mw, cb, opWp, op_b.reshape(N, 1)


def kernel(x, mask, node_emb, gat_W, gat_a, fp_W, fp_b, op_W, op_b, k):
    from concourse import bass_utils

    x = np.asarray(x, np.float32)
    mask = np.asarray(mask, np.float32)
    gat_W = np.asarray(gat_W, np.float32)
    fp_W = np.asarray(fp_W, np.float32)
    fp_b = np.asarray(fp_b, np.float32)
    op_W = np.asarray(op_W, np.float32)
    op_b = np.asarray(op_b, np.float32)

    adj = _build_adjacency_host(node_emb, int(k))
    Mw, cb, opWp, opb2 = _pack_weights(gat_W, fp_W, fp_b, op_W, op_b)

    if "nc" not in _cache:
        _cache["nc"] = _build_bass()
    nc = _cache["nc"]

    xf = x.reshape(BW, N)
    mf = mask.reshape(BW, N)
    uf = xf * (1.0 - mf)
    in_maps = []
    for i in range(NCORES):
        rows = slice(i * ROWS, (i + 1) * ROWS)
        bigin = np.empty((N, C_TOT), np.float32)
        bigin[:, C_XT : C_XT + ROWS] = xf[rows].T
        bigin[:, C_UT : C_UT + ROWS] = uf[rows].T
        bigin[:, C_MT : C_MT + ROWS] = mf[rows].T
        bigin[:, C_CB : C_CB + H] = cb
        bigin[:, C_OPB : C_OPB + 1] = opb2
        bigin[:, C_MW : C_MW + HID] = Mw
        bigin[:, C_OPW : C_OPW + HID] = opWp
        in_maps.append({"bigin": bigin})

    try:
        res = bass_utils.run_bass_kernel_spmd(nc, in_maps, core_ids=list(range(NCORES)))
    except ModuleNotFoundError:
        # BASS_TRACE=1 requires the NTFF hook (antenv.axon_hooks), absent in
        # slim axon containers — retry with tracing forced off.
        import os

        os.environ["BASS_NEVER_TRACE"] = "1"
        res = bass_utils.run_bass_kernel_spmd(nc, in_maps, core_ids=list(range(NCORES)))
    _cache["last_results"] = res

    out = np.empty((BW, N), np.float32)
    for i in range(NCORES):
        out[i * ROWS : (i + 1) * ROWS] = res.results[i]["outT"].T
    return out.reshape(B, W, N), adj


# revision 19
# speedup vs baseline: 1.4003x; 1.4003x over previous
"""Trainium2 Bass kernel for nn_MissingValueImputer (GAT-based imputer).

Mathematical structure of the reference model: the feature projection
broadcasts the *same* projected sample vector to every one of the N graph
nodes, so the per-head node features h[b,h,n,:] do not depend on n. The
additive attention logits e[b,h,i,j] are therefore constant over (i,j),
and the adjacency-masked softmax collapses to attn = adj / k (each top-k
adjacency row has exactly k ones, and softmax of a constant masked row is
uniform). Message passing then yields h' = relu(h * sum_j attn[i,j]) =
relu(h) for every node, so the whole GAT

    imputed = mean_n( concat_heads(h') @ op_W + op_b )

reduces exactly (up to float reassociation) to a 2-layer MLP applied to
each sample vector:

    G       = x_flat @ (fp_W @ Wcat) + fp_b @ Wcat      # [BW, H*F]
    imputed = relu(G) @ op_W + op_b                      # [BW, N]
    output  = x * (1 - mask) + imputed * mask

with Wcat[f, h*F + o] = gat_W[h, o, f]. The adjacency matrix (second
return value) depends only on node_emb and is computed with the exact
reference jax ops on CPU. The MLP + masked mix runs on 8 NeuronCores,
data-parallel over the BW = 512 batch rows (64 rows per core), with all
on-device tensors kept transposed so the partition dimension is 128.

Device layout (per core, everything [128 partitions, cols]):
  inA  [128,133] = xT(64) | maskT(64) | cb(4) | opb(1)   - sync HWDGE ring
  opW  [128,512] = op_W K-chunks side by side             - sync HWDGE ring
  Mw   [128,512] = fp_W @ Wcat                            - scalar HWDGE ring
  GT_j = Mw[:,j*128:...].T @ xT          (PE, psum)
  rT_j = max(GT_j + cb[:,j], 0)          (DVE tensor_scalar, no ACT table)
  impT = sum_j opW[:,j*128:...].T @ rT_j (PE, psum accumulate)
  outT = xT + maskT * (impT + opb - xT)  (DVE x3)
"""

import numpy as np

B, W, N, F, H = 8, 64, 128, 128, 4
BW = B * W            # 512 batch rows
NCORES = 8
ROWS = BW // NCORES   # 64 rows per core
HID = H * F           # 512 hidden units

# packed input layout, all [128, cols]:
#   xT(64) | uT(64) | mT(64) | cb(4) | opb(1) | Mw(512) | opW(512)
# HW-measured: keeping the epilogue tensors (uT, mT) in the FIRST piece beats
# deferring them to the last piece (7.4 vs 11.1 us/iter).
C_XT = 0
C_UT = ROWS
C_MT = 2 * ROWS
C_CB = 3 * ROWS
C_OPB = C_CB + H
C_MW = C_OPB + 1                  # 197
C_OPW = C_MW + HID                # 709
C_TOT = C_OPW + HID               # 1221

# DMA split: list of (engine, col_start, col_end) pieces of the packed input.
# HW-measured best: small piece on sync, Mw on the scalar HWDGE ring, opW on
# sync (8.3us/iter vs 9.5 for two-piece and 10.4 for one-piece).
DEFAULT_SPLIT = (("sync", 0, C_MW), ("scalar", C_MW, C_OPW), ("sync", C_OPW, C_TOT))

_cache = {}


def _build_adjacency_host(node_emb, k):
    """Exact replica of the reference build_adjacency, on jax-CPU."""
    import jax
    import jax.numpy as jnp

    cpu = jax.devices("cpu")[0]
    with jax.default_device(cpu):
        ne = jnp.asarray(np.asarray(node_emb, np.float32))
        norm = ne / jnp.linalg.norm(ne, axis=1, keepdims=True)
        sim = norm @ norm.T
        _, idx = jax.lax.top_k(sim, k)
        n = sim.shape[0]
        rows = jnp.arange(n)[:, None]
        adj = jnp.zeros_like(sim).at[rows, idx].set(1.0)
    return np.asarray(adj)


def _build_bass(iters=1, loop_n=0, split=DEFAULT_SPLIT, out_engine="sync",
                scratch=16384, relu_engine="vector"):
    import concourse.mybir as mybir
    from concourse import bacc
    from concourse.alu_op_type import AluOpType
    from concourse.tile import TileContext

    dt = mybir.dt.float32
    nc = bacc.Bacc(
        "TRN2",
        target_bir_lowering=False,
        debug=False,
        enable_asserts=True,
        num_devices=NCORES,
        dynamic_dma_scratch_size=scratch,
    )

    bigin = nc.dram_tensor("bigin", [N, C_TOT], dt, kind="ExternalInput").ap()
    outT = nc.dram_tensor("outT", [N, ROWS], dt, kind="ExternalOutput").ap()

    with TileContext(nc) as tc:
        with (
            tc.tile_pool(name="sb", bufs=1) as sb,
            tc.tile_pool(name="ps", bufs=1, space="PSUM") as ps,
        ):
            def body(_iv=None):
                big_s = sb.tile([N, C_TOT], dt, tag="big")
                for eng, c0, c1 in split:
                    dma = nc.sync if eng == "sync" else nc.scalar
                    dma.dma_start(out=big_s[:, c0:c1], in_=bigin[:, c0:c1])

                xT_ap = big_s[:, C_XT : C_XT + ROWS]
                uT_ap = big_s[:, C_UT : C_UT + ROWS]
                mT_ap = big_s[:, C_MT : C_MT + ROWS]
                opb_ap = big_s[:, C_OPB : C_OPB + 1]

                imp_ps = ps.tile([N, ROWS], mybir.dt.float32, tag="imp")
                for j in range(H):
                    # GT_j = Mw[:, j-chunk].T @ xT  -> [128 hidden, 64 rows]
                    gT_ps = ps.tile([F, ROWS], mybir.dt.float32, tag=f"g{j}")
                    nc.tensor.matmul(
                        gT_ps[:],
                        big_s[:, C_MW + j * F : C_MW + (j + 1) * F],
                        xT_ap,
                        start=True,
                        stop=True,
                    )
                    # rT_j = max(GT_j + cb_j, 0)
                    rT_s = sb.tile([F, ROWS], dt, tag=f"r{j}")
                    if relu_engine == "vector":
                        nc.vector.tensor_scalar(
                            rT_s[:],
                            gT_ps[:],
                            big_s[:, C_CB + j : C_CB + j + 1],
                            0.0,
                            AluOpType.add,
                            AluOpType.max,
                        )
                    else:
                        nc.scalar.activation(
                            rT_s[:],
                            gT_ps[:],
                            mybir.ActivationFunctionType.Relu,
                            bias=big_s[:, C_CB + j : C_CB + j + 1],
                            scale=1.0,
                        )
                    # impT += opW_j.T @ rT_j  -> [128 out, 64 rows]
                    nc.tensor.matmul(
                        imp_ps[:],
                        big_s[:, C_OPW + j * F : C_OPW + (j + 1) * F],
                        rT_s[:],
                        start=(j == 0),
                        stop=(j == H - 1),
                    )

                # outT = uT + maskT * (impT + opb),  uT = (x*(1-mask)).T
                p_s = sb.tile([N, ROWS], dt, tag="p")
                o_s = sb.tile([N, ROWS], dt, tag="o")
                if out_engine == "split4":
                    q = ROWS // 4
                    for i, odma in enumerate((nc.sync, nc.scalar, nc.sync, nc.scalar)):
                        hs = slice(i * q, (i + 1) * q)
                        nc.vector.scalar_tensor_tensor(
                            p_s[:, hs], imp_ps[:, hs], opb_ap, mT_ap[:, hs],
                            AluOpType.add, AluOpType.mult,
                        )
                        nc.vector.tensor_add(o_s[:, hs], p_s[:, hs], uT_ap[:, hs])
                        odma.dma_start(out=outT[:, hs], in_=o_s[:, hs])
                elif out_engine.startswith("split"):
                    eng = {"split": (nc.sync, nc.scalar),
                           "split-ss": (nc.sync, nc.sync),
                           "split-aa": (nc.scalar, nc.scalar)}[out_engine]
                    halves = ((slice(0, ROWS // 2), eng[0]),
                              (slice(ROWS // 2, ROWS), eng[1]))
                    for hs, odma in halves:
                        nc.vector.scalar_tensor_tensor(
                            p_s[:, hs], imp_ps[:, hs], opb_ap, mT_ap[:, hs],
                            AluOpType.add, AluOpType.mult,
                        )
                        nc.vector.tensor_add(o_s[:, hs], p_s[:, hs], uT_ap[:, hs])
                        odma.dma_start(out=outT[:, hs], in_=o_s[:, hs])
                else:
                    nc.vector.scalar_tensor_tensor(
                        p_s[:], imp_ps[:], opb_ap, mT_ap,
                        AluOpType.add, AluOpType.mult,
                    )
                    nc.vector.tensor_add(o_s[:], p_s[:], uT_ap)
                    odma = nc.sync if out_engine == "sync" else nc.scalar
                    odma.dma_start(out=outT[:], in_=o_s[:])

            if loop_n:
                tc.For_i_unrolled(0, loop_n, 1, body, max_unroll=8)
            else:
                for _ in range(iters):
                    body()

    nc.compile()
    return nc


def _pack_weights(gat_W, fp_W, fp_b, op_W, op_b):
    # Wcat[f, h*F+o] = gat_W[h, o, f]; fold the (linear) feature projection in.
    Wcat = np.ascontiguousarray(np.transpose(gat_W, (2, 0, 1)).reshape(F, HID))
    Mw = np.ascontiguousarray(fp_W @ Wcat)                 # [128, 512]
    c = fp_b @ Wcat                                        # [512]
    cb = np.ascontiguousarray(c.reshape(H, F).T)           # [128, 4]
    # opW packed so that col block j holds op_W rows j*128:(j+1)*128
    opWp = np.ascontiguousarray(
        op_W.reshape(H, F, N).transpose(1, 0, 2).reshape(F, HID)
    )
    return Mw, cb, opWp, op_b.reshape(N, 1)


def kernel(x, mask, node_emb, gat_W, gat_a, fp_W, fp_b, op_W, op_b, k):
    from concourse import bass_utils

    x = np.asarray(x, np.float32)
    mask = np.asarray(mask, np.float32)
    gat_W = np.asarray(gat_W, np.float32)
    fp_W = np.asarray(fp_W, np.float32)
    fp_b = np.asarray(fp_b, np.float32)
    op_W = np.asarray(op_W, np.float32)
    op_b = np.asarray(op_b, np.float32)

    adj = _build_adjacency_host(node_emb, int(k))
    Mw, cb, opWp, opb2 = _pack_weights(gat_W, fp_W, fp_b, op_W, op_b)

    if "nc" not in _cache:
        _cache["nc"] = _build_bass()
    nc = _cache["nc"]

    xf = x.reshape(BW, N)
    mf = mask.reshape(BW, N)
    uf = xf * (1.0 - mf)
    in_maps = []
    for i in range(NCORES):
        rows = slice(i * ROWS, (i + 1) * ROWS)
        bigin = np.empty((N, C_TOT), np.float32)
        bigin[:, C_XT : C_XT + ROWS] = xf[rows].T
        bigin[:, C_UT : C_UT + ROWS] = uf[rows].T
        bigin[:, C_MT : C_MT + ROWS] = mf[rows].T
        bigin[:, C_CB : C_CB + H] = cb
        bigin[:, C_OPB : C_OPB + 1] = opb2
        bigin[:, C_MW : C_MW + HID] = Mw
        bigin[:, C_OPW : C_OPW + HID] = opWp
        in_maps.append({"bigin": bigin})

    try:
        res = bass_utils.run_bass_kernel_spmd(nc, in_maps, core_ids=list(range(NCORES)))
    except ModuleNotFoundError:
        # BASS_TRACE=1 requires the NTFF hook (antenv.axon_hooks), absent in
        # slim axon containers — retry with tracing forced off.
        import os

        os.environ["BASS_NEVER_TRACE"] = "1"
        res = bass_utils.run_bass_kernel_spmd(nc, in_maps, core_ids=list(range(NCORES)))
    _cache["last_results"] = res

    out = np.empty((BW, N), np.float32)
    for i in range(NCORES):
        out[i * ROWS : (i + 1) * ROWS] = res.results[i]["outT"].T
    return out.reshape(B, W, N), adj


# revision 21
# speedup vs baseline: 1.4085x; 1.0059x over previous
"""Trainium2 Bass kernel for nn_MissingValueImputer (GAT-based imputer).

Mathematical structure of the reference model: the feature projection
broadcasts the *same* projected sample vector to every one of the N graph
nodes, so the per-head node features h[b,h,n,:] do not depend on n. The
additive attention logits e[b,h,i,j] are therefore constant over (i,j),
and the adjacency-masked softmax collapses to attn = adj / k (each top-k
adjacency row has exactly k ones, and softmax of a constant masked row is
uniform). Message passing then yields h' = relu(h * sum_j attn[i,j]) =
relu(h) for every node, so the whole GAT

    imputed = mean_n( concat_heads(h') @ op_W + op_b )

reduces exactly (up to float reassociation) to a 2-layer MLP applied to
each sample vector:

    G       = x_flat @ (fp_W @ Wcat) + fp_b @ Wcat      # [BW, H*F]
    imputed = relu(G) @ op_W + op_b                      # [BW, N]
    output  = x * (1 - mask) + imputed * mask

with Wcat[f, h*F + o] = gat_W[h, o, f]. The adjacency matrix (second
return value) depends only on node_emb and is computed with the exact
reference jax ops on CPU. The MLP + masked mix runs on 8 NeuronCores,
data-parallel over the BW = 512 batch rows (64 rows per core), with all
on-device tensors kept transposed so the partition dimension is 128.

Device pipeline (per core, everything [128 partitions, cols], one packed
input tensor DMAed in 3 pieces across the two HWDGE rings — HW-measured
best split, ~8.4us/exec vs ~10.5 for the naive 11-DMA version):
  GT_j = Mw[:,j*128:...].T @ xT          (PE, psum)
  rT_j = max(GT_j + cb[:,j], 0)          (DVE tensor_scalar, no ACT table)
  impT = sum_j opW[:,j*128:...].T @ rT_j (PE, psum accumulate)
  outT = uT + maskT * (impT + opb)       (DVE x2, uT = (x*(1-mask)).T)
"""

import numpy as np

B, W, N, F, H = 8, 64, 128, 128, 4
BW = B * W            # 512 batch rows
NCORES = 8
ROWS = BW // NCORES   # 64 rows per core
HID = H * F           # 512 hidden units

# packed input layout, all [128, cols]:
#   xT(64) | uT(64) | mT(64) | cb(4) | opb(1) | Mw(512) | opW(512)
# HW-measured: keeping the epilogue tensors (uT, mT) in the FIRST piece beats
# deferring them to the last piece (7.4 vs 11.1 us/iter).
C_XT = 0
C_UT = ROWS
C_MT = 2 * ROWS
C_CB = 3 * ROWS
C_OPB = C_CB + H
C_MW = C_OPB + 1                  # 197
C_OPW = C_MW + HID                # 709
C_TOT = C_OPW + HID               # 1221

# DMA split: list of (engine, col_start, col_end) pieces of the packed input.
# HW-measured best: small piece on sync, Mw on the scalar HWDGE ring, opW on
# sync (8.3us/iter vs 9.5 for two-piece and 10.4 for one-piece).
DEFAULT_SPLIT = (("sync", 0, C_MW), ("scalar", C_MW, C_OPW), ("sync", C_OPW, C_TOT))

_cache = {}


def _build_adjacency_host(node_emb, k):
    """Exact replica of the reference build_adjacency, on jax-CPU."""
    import jax
    import jax.numpy as jnp

    cpu = jax.devices("cpu")[0]
    with jax.default_device(cpu):
        ne = jnp.asarray(np.asarray(node_emb, np.float32))
        norm = ne / jnp.linalg.norm(ne, axis=1, keepdims=True)
        sim = norm @ norm.T
        _, idx = jax.lax.top_k(sim, k)
        n = sim.shape[0]
        rows = jnp.arange(n)[:, None]
        adj = jnp.zeros_like(sim).at[rows, idx].set(1.0)
    return np.asarray(adj)


def _build_bass(iters=1, loop_n=0, split=DEFAULT_SPLIT, out_engine="sync",
                scratch=16384, relu_engine="vector"):
    import concourse.mybir as mybir
    from concourse import bacc
    from concourse.alu_op_type import AluOpType
    from concourse.tile import TileContext

    dt = mybir.dt.float32
    nc = bacc.Bacc(
        "TRN2",
        target_bir_lowering=False,
        debug=False,
        enable_asserts=True,
        num_devices=NCORES,
        dynamic_dma_scratch_size=scratch,
    )

    bigin = nc.dram_tensor("bigin", [N, C_TOT], dt, kind="ExternalInput").ap()
    outT = nc.dram_tensor("outT", [N, ROWS], dt, kind="ExternalOutput").ap()

    with TileContext(nc) as tc:
        with (
            tc.tile_pool(name="sb", bufs=1) as sb,
            tc.tile_pool(name="ps", bufs=1, space="PSUM") as ps,
        ):
            def body(_iv=None):
                big_s = sb.tile([N, C_TOT], dt, tag="big")
                for eng, c0, c1 in split:
                    dma = nc.sync if eng == "sync" else nc.scalar
                    dma.dma_start(out=big_s[:, c0:c1], in_=bigin[:, c0:c1])

                xT_ap = big_s[:, C_XT : C_XT + ROWS]
                uT_ap = big_s[:, C_UT : C_UT + ROWS]
                mT_ap = big_s[:, C_MT : C_MT + ROWS]
                opb_ap = big_s[:, C_OPB : C_OPB + 1]

                imp_ps = ps.tile([N, ROWS], mybir.dt.float32, tag="imp")
                for j in range(H):
                    # GT_j = Mw[:, j-chunk].T @ xT  -> [128 hidden, 64 rows]
                    gT_ps = ps.tile([F, ROWS], mybir.dt.float32, tag=f"g{j}")
                    nc.tensor.matmul(
                        gT_ps[:],
                        big_s[:, C_MW + j * F : C_MW + (j + 1) * F],
                        xT_ap,
                        start=True,
                        stop=True,
                    )
                    # rT_j = max(GT_j + cb_j, 0)
                    rT_s = sb.tile([F, ROWS], dt, tag=f"r{j}")
                    if relu_engine == "vector":
                        nc.vector.tensor_scalar(
                            rT_s[:],
                            gT_ps[:],
                            big_s[:, C_CB + j : C_CB + j + 1],
                            0.0,
                            AluOpType.add,
                            AluOpType.max,
                        )
                    else:
                        nc.scalar.activation(
                            rT_s[:],
                            gT_ps[:],
                            mybir.ActivationFunctionType.Relu,
                            bias=big_s[:, C_CB + j : C_CB + j + 1],
                            scale=1.0,
                        )
                    # impT += opW_j.T @ rT_j  -> [128 out, 64 rows]
                    nc.tensor.matmul(
                        imp_ps[:],
                        big_s[:, C_OPW + j * F : C_OPW + (j + 1) * F],
                        rT_s[:],
                        start=(j == 0),
                        stop=(j == H - 1),
                    )

                # outT = uT + maskT * (impT + opb),  uT = (x*(1-mask)).T
                p_s = sb.tile([N, ROWS], dt, tag="p")
                o_s = sb.tile([N, ROWS], dt, tag="o")
                if out_engine == "split":
                    halves = ((slice(0, ROWS // 2), nc.sync),
                              (slice(ROWS // 2, ROWS), nc.scalar))
                    for hs, odma in halves:
                        nc.vector.scalar_tensor_tensor(
                            p_s[:, hs], imp_ps[:, hs], opb_ap, mT_ap[:, hs],
                            AluOpType.add, AluOpType.mult,
                        )
                        nc.vector.tensor_add(o_s[:, hs], p_s[:, hs], uT_ap[:, hs])
                        odma.dma_start(out=outT[:, hs], in_=o_s[:, hs])
                else:
                    nc.vector.scalar_tensor_tensor(
                        p_s[:], imp_ps[:], opb_ap, mT_ap,
                        AluOpType.add, AluOpType.mult,
                    )
                    nc.vector.tensor_add(o_s[:], p_s[:], uT_ap)
                    odma = nc.sync if out_engine == "sync" else nc.scalar
                    odma.dma_start(out=outT[:], in_=o_s[:])

            if loop_n:
                tc.For_i_unrolled(0, loop_n, 1, body, max_unroll=8)
            else:
                for _ in range(iters):
                    body()

    nc.compile()
    return nc


def _pack_weights(gat_W, fp_W, fp_b, op_W, op_b):
    # Wcat[f, h*F+o] = gat_W[h, o, f]; fold the (linear) feature projection in.
    Wcat = np.ascontiguousarray(np.transpose(gat_W, (2, 0, 1)).reshape(F, HID))
    Mw = np.ascontiguousarray(fp_W @ Wcat)                 # [128, 512]
    c = fp_b @ Wcat                                        # [512]
    cb = np.ascontiguousarray(c.reshape(H, F).T)           # [128, 4]
    # opW packed so that col block j holds op_W rows j*128:(j+1)*128
    opWp = np.ascontiguousarray(
        op_W.reshape(H, F, N).transpose(1, 0, 2).reshape(F, HID)
    )
    return Mw, cb, opWp, op_b.reshape(N, 1)


def kernel(x, mask, node_emb, gat_W, gat_a, fp_W, fp_b, op_W, op_b, k):
    from concourse import bass_utils

    x = np.asarray(x, np.float32)
    mask = np.asarray(mask, np.float32)
    gat_W = np.asarray(gat_W, np.float32)
    fp_W = np.asarray(fp_W, np.float32)
    fp_b = np.asarray(fp_b, np.float32)
    op_W = np.asarray(op_W, np.float32)
    op_b = np.asarray(op_b, np.float32)

    adj = _build_adjacency_host(node_emb, int(k))
    Mw, cb, opWp, opb2 = _pack_weights(gat_W, fp_W, fp_b, op_W, op_b)

    if "nc" not in _cache:
        _cache["nc"] = _build_bass()
    nc = _cache["nc"]

    xf = x.reshape(BW, N)
    mf = mask.reshape(BW, N)
    uf = xf * (1.0 - mf)
    in_maps = []
    for i in range(NCORES):
        rows = slice(i * ROWS, (i + 1) * ROWS)
        bigin = np.empty((N, C_TOT), np.float32)
        bigin[:, C_XT : C_XT + ROWS] = xf[rows].T
        bigin[:, C_UT : C_UT + ROWS] = uf[rows].T
        bigin[:, C_MT : C_MT + ROWS] = mf[rows].T
        bigin[:, C_CB : C_CB + H] = cb
        bigin[:, C_OPB : C_OPB + 1] = opb2
        bigin[:, C_MW : C_MW + HID] = Mw
        bigin[:, C_OPW : C_OPW + HID] = opWp
        in_maps.append({"bigin": bigin})

    try:
        res = bass_utils.run_bass_kernel_spmd(nc, in_maps, core_ids=list(range(NCORES)))
    except ModuleNotFoundError:
        # BASS_TRACE=1 requires the NTFF hook (antenv.axon_hooks), absent in
        # slim axon containers — retry with tracing forced off.
        import os

        os.environ["BASS_NEVER_TRACE"] = "1"
        res = bass_utils.run_bass_kernel_spmd(nc, in_maps, core_ids=list(range(NCORES)))
    _cache["last_results"] = res

    out = np.empty((BW, N), np.float32)
    for i in range(NCORES):
        out[i * ROWS : (i + 1) * ROWS] = res.results[i]["outT"].T
    return out.reshape(B, W, N), adj


# revision 28
# speedup vs baseline: 1.4394x; 1.0219x over previous
"""Trainium2 Bass kernel for nn_MissingValueImputer (GAT-based imputer).

Mathematical structure of the reference model: the feature projection
broadcasts the *same* projected sample vector to every one of the N graph
nodes, so the per-head node features h[b,h,n,:] do not depend on n. The
additive attention logits e[b,h,i,j] are therefore constant over (i,j),
and the adjacency-masked softmax collapses to attn = adj / k (each top-k
adjacency row has exactly k ones, and softmax of a constant masked row is
uniform). Message passing then yields h' = relu(h * sum_j attn[i,j]) =
relu(h) for every node, so the whole GAT

    imputed = mean_n( concat_heads(h') @ op_W + op_b )

reduces exactly (up to float reassociation) to a 2-layer MLP applied to
each sample vector:

    G       = x_flat @ (fp_W @ Wcat) + fp_b @ Wcat      # [BW, H*F]
    imputed = relu(G) @ op_W + op_b                      # [BW, N]
    output  = x * (1 - mask) + imputed * mask

with Wcat[f, h*F + o] = gat_W[h, o, f]. The adjacency matrix (second
return value) depends only on node_emb and is computed with the exact
reference jax ops on CPU. The MLP + masked mix runs on 8 NeuronCores,
data-parallel over the BW = 512 batch rows (64 rows per core), with all
on-device tensors kept transposed so the partition dimension is 128.

Device pipeline (per core, everything [128 partitions, cols], one packed
input tensor DMAed in 3 pieces across the two HWDGE rings — HW-measured
best split, ~8.4us/exec vs ~10.5 for the naive 11-DMA version):
  GT_j = Mw[:,j*128:...].T @ xT          (PE, psum)
  rT_j = max(GT_j + cb[:,j], 0)          (DVE tensor_scalar, no ACT table)
  impT = sum_j opW[:,j*128:...].T @ rT_j (PE, psum accumulate)
  outT = uT + maskT * (impT + opb)       (DVE x2, uT = (x*(1-mask)).T)
"""

import numpy as np

B, W, N, F, H = 8, 64, 128, 128, 4
BW = B * W            # 512 batch rows
NCORES = 8
ROWS = BW // NCORES   # 64 rows per core
HID = H * F           # 512 hidden units

# packed input layout, all [128, cols]:
#   xT(64) | uT(64) | mT(64) | cb(4) | opb(1) | Mw(512) | opW(512)
# HW-measured: keeping the epilogue tensors (uT, mT) in the FIRST piece beats
# deferring them to the last piece by ~3 us/iter.
C_XT = 0
C_UT = ROWS
C_MT = 2 * ROWS
C_CB = 3 * ROWS
C_OPB = C_CB + H
C_MW = C_OPB + 1                  # 197
C_OPW = C_MW + HID                # 709
C_TOT = C_OPW + HID               # 1221

# DMA split: list of (engine, col_start, col_end) pieces of the packed input.
# HW-measured best: small piece on sync, Mw on the scalar HWDGE ring, opW on
# sync (8.3us/iter vs 9.5 for two-piece and 10.4 for one-piece).
DEFAULT_SPLIT = (("sync", 0, C_MW), ("scalar", C_MW, C_OPW), ("sync", C_OPW, C_TOT))

_cache = {}


def _build_adjacency_host(node_emb, k):
    """Exact replica of the reference build_adjacency, on jax-CPU."""
    import jax
    import jax.numpy as jnp

    cpu = jax.devices("cpu")[0]
    with jax.default_device(cpu):
        ne = jnp.asarray(np.asarray(node_emb, np.float32))
        norm = ne / jnp.linalg.norm(ne, axis=1, keepdims=True)
        sim = norm @ norm.T
        _, idx = jax.lax.top_k(sim, k)
        n = sim.shape[0]
        rows = jnp.arange(n)[:, None]
        adj = jnp.zeros_like(sim).at[rows, idx].set(1.0)
    return np.asarray(adj)


def _build_bass(iters=1, loop_n=0, split=DEFAULT_SPLIT, out_engine="sync",
                scratch=16384, relu_engine="vector", opw0_mode="none",
                loop_unroll=8):
    import concourse.mybir as mybir
    from concourse import bacc
    from concourse.alu_op_type import AluOpType
    from concourse.tile import TileContext

    # opw0_mode relocates op_W chunk 0 into an earlier DMA piece so the PSUM
    # accumulation chain is not gated on the last piece's completion:
    #   "p1": xT..opb | opW0 | Mw | opW123   (opW0 rides the sync small piece)
    #   "p2": xT..opb | Mw | opW0 | opW123   (opW0 rides the scalar Mw piece)
    if opw0_mode == "p1":
        mw0, opw_col = C_MW + F, lambda j: C_MW if j == 0 else C_MW + F + HID + (j - 1) * F
        split = (("sync", 0, C_MW + F), ("scalar", C_MW + F, C_MW + F + HID),
                 ("sync", C_MW + F + HID, C_TOT))
    elif opw0_mode == "p2":
        mw0 = C_MW
        opw_col = lambda j: (C_MW + HID + F + (j - 1) * F) if j else C_MW + HID
        split = (("sync", 0, C_MW), ("scalar", C_MW, C_MW + HID + F),
                 ("sync", C_MW + HID + F, C_TOT))
    else:
        mw0, opw_col = C_MW, lambda j: C_OPW + j * F

    dt = mybir.dt.float32
    nc = bacc.Bacc(
        "TRN2",
        target_bir_lowering=False,
        debug=False,
        enable_asserts=True,
        num_devices=NCORES,
        dynamic_dma_scratch_size=scratch,
    )

    bigin = nc.dram_tensor("bigin", [N, C_TOT], dt, kind="ExternalInput").ap()
    outT = nc.dram_tensor("outT", [N, ROWS], dt, kind="ExternalOutput").ap()

    with TileContext(nc) as tc:
        with (
            tc.tile_pool(name="sb", bufs=1) as sb,
            tc.tile_pool(name="ps", bufs=1, space="PSUM") as ps,
        ):
            def body(_iv=None):
                big_s = sb.tile([N, C_TOT], dt, tag="big")
                for eng, c0, c1 in split:
                    dma = nc.sync if eng == "sync" else nc.scalar
                    dma.dma_start(out=big_s[:, c0:c1], in_=bigin[:, c0:c1])

                xT_ap = big_s[:, C_XT : C_XT + ROWS]
                uT_ap = big_s[:, C_UT : C_UT + ROWS]
                mT_ap = big_s[:, C_MT : C_MT + ROWS]
                opb_ap = big_s[:, C_OPB : C_OPB + 1]

                imp_ps = ps.tile([N, ROWS], mybir.dt.float32, tag="imp")
                for j in range(H):
                    # GT_j = Mw[:, j-chunk].T @ xT  -> [128 hidden, 64 rows]
                    gT_ps = ps.tile([F, ROWS], mybir.dt.float32, tag=f"g{j}")
                    nc.tensor.matmul(
                        gT_ps[:],
                        big_s[:, mw0 + j * F : mw0 + (j + 1) * F],
                        xT_ap,
                        start=True,
                        stop=True,
                    )
                    # rT_j = max(GT_j + cb_j, 0)
                    rT_s = sb.tile([F, ROWS], dt, tag=f"r{j}")
                    if relu_engine == "vector":
                        nc.vector.tensor_scalar(
                            rT_s[:],
                            gT_ps[:],
                            big_s[:, C_CB + j : C_CB + j + 1],
                            0.0,
                            AluOpType.add,
                            AluOpType.max,
                        )
                    else:
                        nc.scalar.activation(
                            rT_s[:],
                            gT_ps[:],
                            mybir.ActivationFunctionType.Relu,
                            bias=big_s[:, C_CB + j : C_CB + j + 1],
                            scale=1.0,
                        )
                    # impT += opW_j.T @ rT_j  -> [128 out, 64 rows]
                    nc.tensor.matmul(
                        imp_ps[:],
                        big_s[:, opw_col(j) : opw_col(j) + F],
                        rT_s[:],
                        start=(j == 0),
                        stop=(j == H - 1),
                    )

                # outT = uT + maskT * (impT + opb),  uT = (x*(1-mask)).T
                p_s = sb.tile([N, ROWS], dt, tag="p")
                o_s = sb.tile([N, ROWS], dt, tag="o")
                if out_engine == "split":
                    halves = ((slice(0, ROWS // 2), nc.sync),
                              (slice(ROWS // 2, ROWS), nc.scalar))
                    for hs, odma in halves:
                        nc.vector.scalar_tensor_tensor(
                            p_s[:, hs], imp_ps[:, hs], opb_ap, mT_ap[:, hs],
                            AluOpType.add, AluOpType.mult,
                        )
                        nc.vector.tensor_add(o_s[:, hs], p_s[:, hs], uT_ap[:, hs])
                        odma.dma_start(out=outT[:, hs], in_=o_s[:, hs])
                else:
                    nc.vector.scalar_tensor_tensor(
                        p_s[:], imp_ps[:], opb_ap, mT_ap,
                        AluOpType.add, AluOpType.mult,
                    )
                    nc.vector.tensor_add(o_s[:], p_s[:], uT_ap)
                    odma = nc.sync if out_engine == "sync" else nc.scalar
                    odma.dma_start(out=outT[:], in_=o_s[:])

            if loop_n:
                tc.For_i_unrolled(0, loop_n, 1, body, max_unroll=loop_unroll)
            else:
                for _ in range(iters):
                    body()

    nc.compile()
    return nc


def _pack_weights(gat_W, fp_W, fp_b, op_W, op_b):
    # Wcat[f, h*F+o] = gat_W[h, o, f]; fold the (linear) feature projection in.
    Wcat = np.ascontiguousarray(np.transpose(gat_W, (2, 0, 1)).reshape(F, HID))
    Mw = np.ascontiguousarray(fp_W @ Wcat)                 # [128, 512]
    c = fp_b @ Wcat                                        # [512]
    cb = np.ascontiguousarray(c.reshape(H, F).T)           # [128, 4]
    # opW packed so that col block j holds op_W rows j*128:(j+1)*128
    opWp = np.ascontiguousarray(
        op_W.reshape(H, F, N).transpose(1, 0, 2).reshape(F, HID)
    )
    return Mw, cb, opWp, op_b.reshape(N, 1)


def kernel(x, mask, node_emb, gat_W, gat_a, fp_W, fp_b, op_W, op_b, k):
    from concourse import bass_utils

    x = np.asarray(x, np.float32)
    mask = np.asarray(mask, np.float32)
    gat_W = np.asarray(gat_W, np.float32)
    fp_W = np.asarray(fp_W, np.float32)
    fp_b = np.asarray(fp_b, np.float32)
    op_W = np.asarray(op_W, np.float32)
    op_b = np.asarray(op_b, np.float32)

    adj = _build_adjacency_host(node_emb, int(k))
    Mw, cb, opWp, opb2 = _pack_weights(gat_W, fp_W, fp_b, op_W, op_b)

    if "nc" not in _cache:
        _cache["nc"] = _build_bass()
    nc = _cache["nc"]

    xf = x.reshape(BW, N)
    mf = mask.reshape(BW, N)
    uf = xf * (1.0 - mf)
    in_maps = []
    for i in range(NCORES):
        rows = slice(i * ROWS, (i + 1) * ROWS)
        bigin = np.empty((N, C_TOT), np.float32)
        bigin[:, C_XT : C_XT + ROWS] = xf[rows].T
        bigin[:, C_UT : C_UT + ROWS] = uf[rows].T
        bigin[:, C_MT : C_MT + ROWS] = mf[rows].T
        bigin[:, C_CB : C_CB + H] = cb
        bigin[:, C_OPB : C_OPB + 1] = opb2
        bigin[:, C_MW : C_MW + HID] = Mw
        bigin[:, C_OPW : C_OPW + HID] = opWp
        in_maps.append({"bigin": bigin})

    try:
        res = bass_utils.run_bass_kernel_spmd(nc, in_maps, core_ids=list(range(NCORES)))
    except ModuleNotFoundError:
        # BASS_TRACE=1 requires the NTFF hook (antenv.axon_hooks), absent in
        # slim axon containers — retry with tracing forced off.
        import os

        os.environ["BASS_NEVER_TRACE"] = "1"
        res = bass_utils.run_bass_kernel_spmd(nc, in_maps, core_ids=list(range(NCORES)))
    _cache["last_results"] = res

    out = np.empty((BW, N), np.float32)
    for i in range(NCORES):
        out[i * ROWS : (i + 1) * ROWS] = res.results[i]["outT"].T
    return out.reshape(B, W, N), adj
